# revision 8
# baseline (speedup 1.0000x reference)
"""Trainium2 Bass kernel for nn_AttentionBasedConvLSTM.

Model: per-(b,t,c) SE-style channel attention gates x, then a 2-layer
ConvLSTM (hidden 64, 3x3 SAME convs) over T=16 steps on 64x64 frames.
Returns (out1, h1, c1) like the reference.

Sharding: data-parallel over batch B=8 across the 8 NeuronCores (one
batch element per core); weights replicated. Recurrent convs run as
9-tap accumulating matmuls on the tensor engine in float32r (~1e-4
rounding, full rate), gates via ScalarE sigmoid/tanh (sigmoid(o) is
0.5*tanh(o/2)+0.5 so one ACT op covers the mixed [g;o] half), cell/h
updates on VectorE in fp32.

Layouts (per core, SBUF):
  HH0 [128, 66*66] f32r: rows 0-63 h0 padded, rows 64-66 gated x_t
      padded -> layer-0 conv rhs is HH0[0:67] at 9 tap offsets (K=67
      keeps PE array occupancy >50% so the HAM clock stays at 2.4GHz).
  HH1 [128, 66*66] f32r: rows 0-63 h0(t), rows 64-127 h1(t-1)
      -> layer-1 conv rhs is HH1[0:128] (K=128).
  C0/C1 [128, 4096] fp32: rows 64-127 hold the cell state (base-64 so
      every elementwise op is partition-base aligned with the f/o gates).

Within a (t, layer): ALL conv matmuls are emitted before any h-write
(program order defines read-after-write semantics on the single-buffered
HH state); Tile's byte-range WAR tracking then orders the h-writes after
the tap reads of the neighboring rows.
"""
import os
import sys

sys.path.insert(0, "/opt/trn_rl_repo")

import numpy as np

import concourse.mybir as mybir
import concourse.tile as tile
from concourse import bacc
from concourse.bass_utils import run_bass_kernel_spmd

F32 = mybir.dt.float32
F32R = mybir.dt.float32r
AF = mybir.ActivationFunctionType

B, T, C, H, W = 8, 16, 3, 64, 64
HID = 64
HP, WP = H + 2, W + 2          # zero-padded frame
PADN = HP * WP
PIX = H * W                    # 4096
NCHUNK = 2                     # conv/psum chunks per frame (2048 px)
CHPIX = PIX // NCHUNK
CHROWS = H // NCHUNK           # 32 image rows per chunk
NSUB = CHPIX // 512            # 512-px matmul slices per chunk

_CACHE = {}


def _build_nc():
    nc = bacc.Bacc("TRN2", target_bir_lowering=False)

    x48 = nc.dram_tensor("x48", (T * C, PIX), F32, kind="ExternalInput")
    w0 = nc.dram_tensor("w0", (67, 9, 256), F32R, kind="ExternalInput")
    w1 = nc.dram_tensor("w1", (128, 9, 256), F32R, kind="ExternalInput")
    bif0 = nc.dram_tensor("bif0", (128, 1), F32, kind="ExternalInput")
    bgo0 = nc.dram_tensor("bgo0", (128, 1), F32, kind="ExternalInput")
    bif1 = nc.dram_tensor("bif1", (128, 1), F32, kind="ExternalInput")
    bgo1 = nc.dram_tensor("bgo1", (128, 1), F32, kind="ExternalInput")
    w1m = nc.dram_tensor("w1m", (112, 32), F32, kind="ExternalInput")
    w2m = nc.dram_tensor("w2m", (32, 48), F32, kind="ExternalInput")
    out1 = nc.dram_tensor("out1", (T, HID, PIX), F32, kind="ExternalOutput")
    c1o = nc.dram_tensor("c1o", (HID, PIX), F32, kind="ExternalOutput")

    with tile.TileContext(nc) as tc:
        with tc.tile_pool(name="const", bufs=1) as const, \
             tc.tile_pool(name="state", bufs=1) as state, \
             tc.tile_pool(name="work", bufs=2) as work, \
             tc.tile_pool(name="scr", bufs=1) as scr, \
             tc.tile_pool(name="psum", bufs=2, space="PSUM") as psum:

            # ---- loads (x first: the attention chain gates everything) --
            X48 = const.tile([T * C, PIX], F32)
            nc.sync.dma_start(X48[:], x48[:, :])
            W0 = const.tile([67, 9, 256], F32R)
            W1 = const.tile([128, 9, 256], F32R)
            nc.sync.dma_start(W0[:], w0[:, :, :])
            nc.sync.dma_start(W1[:], w1[:, :, :])
            BIF0 = const.tile([128, 1], F32)
            BGO0 = const.tile([128, 1], F32)
            BIF1 = const.tile([128, 1], F32)
            BGO1 = const.tile([128, 1], F32)
            nc.scalar.dma_start(BIF0[:], bif0[:, :])
            nc.scalar.dma_start(BGO0[:], bgo0[:, :])
            nc.scalar.dma_start(BIF1[:], bif1[:, :])
            nc.scalar.dma_start(BGO1[:], bgo1[:, :])
            W1M = const.tile([112, 32], F32)
            W2M = const.tile([32, 48], F32)
            nc.scalar.dma_start(W1M[:], w1m[:, :])
            nc.scalar.dma_start(W2M[:], w2m[:, :])

            SCLV = const.tile([128, 1], F32)      # per-partition tanh scale
            nc.vector.memset(SCLV[0:64, :], 1.0)  # g rows
            nc.vector.memset(SCLV[64:128, :], 0.5)  # o rows
            ONES = const.tile([1, 1], F32)
            nc.vector.memset(ONES[:], 1.0)

            # ---- state -------------------------------------------------
            HH0 = state.tile([128, PADN], F32R)
            HH1 = state.tile([128, PADN], F32R)
            C0 = state.tile([128, PIX], F32)
            C1 = state.tile([128, PIX], F32)
            nc.vector.memset(HH0[:].bitcast(F32), 0.0)
            nc.vector.memset(HH1[:].bitcast(F32), 0.0)
            nc.vector.memset(C0[:], 0.0)
            nc.vector.memset(C1[:], 0.0)
            HH0v = HH0.rearrange("p (a b) -> p a b", a=HP)
            HH1v = HH1.rearrange("p (a b) -> p a b", a=HP)

            # ---- attention: scale = sigmoid(fc(avg) + fc(max)) ---------
            V = const.tile([112, 1], F32)
            nc.vector.memset(V[:], 0.0)
            nc.vector.tensor_reduce(V[0:48, :], X48[:, :],
                                    axis=mybir.AxisListType.XYZW,
                                    op=mybir.AluOpType.add)
            nc.vector.tensor_reduce(V[64:112, :], X48[:, :],
                                    axis=mybir.AxisListType.XYZW,
                                    op=mybir.AluOpType.max)
            pa = psum.tile([128, CHPIX], F32, tag="gates", name="pa")
            nc.tensor.matmul(pa[0:1, 0:32], V[:, :], W1M[:, :],
                             start=True, stop=True)
            R = const.tile([1, 32], F32)
            nc.scalar.activation(R[:], pa[0:1, 0:32], AF.Relu)
            pb = psum.tile([128, CHPIX], F32, tag="gates", name="pb")
            nc.tensor.matmul(pb[0:32, 0:1], R[:, :], ONES[:, :],
                             start=True, stop=True)
            R2 = const.tile([32, 1], F32)
            nc.vector.tensor_copy(R2[:], pb[0:32, 0:1])
            pc = psum.tile([128, CHPIX], F32, tag="gates", name="pc")
            nc.tensor.matmul(pc[0:48, 0:1], W2M[:, :], R2[:, :],
                             start=True, stop=True)
            SCL = const.tile([48, 1], F32)
            nc.scalar.activation(SCL[:], pc[0:48, 0:1], AF.Sigmoid)
            XIN = const.tile([48, PIX], F32R)
            nc.vector.tensor_scalar(XIN[:], X48[:, :], SCL[:, 0:1], None,
                                    mybir.AluOpType.mult)
            XINv = XIN.rearrange("p (a b) -> p a b", a=H)

            def write_x(t):
                # gated x_t -> HH0 rows 64:67 padded interior
                nc.sync.dma_start(HH0v[64:67, 1:H + 1, 1:W + 1],
                                  XINv[3 * t:3 * t + 3, :, :])

            write_x(0)

            # ---- recurrence --------------------------------------------
            layers = (
                (HH0v, 67, W0, BIF0, BGO0, C0),
                (HH1v, 128, W1, BIF1, BGO1, C1),
            )
            for t in range(T):
                for L in (0, 1):
                    HHtv, K, Wl, Bif, Bgo, Cc = layers[L]
                    # Phase 1: ALL conv matmuls (both chunks) + gate ACTs.
                    sigs, tts = [], []
                    for ch in range(NCHUNK):
                        ps0 = psum.tile([128, CHPIX], F32, tag="gates",
                                        name=f"ps0_{t}_{L}_{ch}")
                        ps1 = psum.tile([128, CHPIX], F32, tag="gates",
                                        name=f"ps1_{t}_{L}_{ch}")
                        for half, ps in ((0, ps0), (1, ps1)):
                            hs = slice(128 * half, 128 * (half + 1))
                            for tap in range(9):
                                dy, dx = tap // 3, tap % 3
                                for sub in range(NSUB):
                                    r0 = CHROWS * ch + 8 * sub
                                    rhs = HHtv[0:K, r0 + dy:r0 + dy + 8,
                                               dx:dx + W]
                                    nc.tensor.matmul(
                                        ps[:, 512 * sub:512 * (sub + 1)],
                                        Wl[0:K, tap, hs],
                                        rhs, start=(tap == 0),
                                        stop=(tap == 8))
                        # gates: ps0=[i;f] sigmoid; ps1=[g;o] tanh w/ scale
                        SIG = work.tile([128, CHPIX], F32, tag="sig",
                                        name=f"sig_{t}_{L}_{ch}")
                        TT = work.tile([128, CHPIX], F32, tag="tt",
                                       name=f"tt_{t}_{L}_{ch}")
                        nc.scalar.activation(SIG[:], ps0[:], AF.Sigmoid,
                                             bias=Bif[:, 0:1])
                        nc.scalar.activation(TT[:], ps1[:], AF.Tanh,
                                             bias=Bgo[:, 0:1],
                                             scale=SCLV[:, 0:1])
                        sigs.append(SIG)
                        tts.append(TT)
                    # Phase 2: cell/h updates in 1024-px quarters so h
                    # lands progressively (shortens the PE boundary stall).
                    NQ = 4
                    QPIX = PIX // NQ
                    QROWS = H // NQ
                    for q in range(NQ):
                        ch, qq = q // 2, q % 2
                        SIG, TT = sigs[ch], tts[ch]
                        sl = slice(QPIX * qq, QPIX * (qq + 1))
                        csl = Cc[64:128, QPIX * q:QPIX * (q + 1)]
                        t2 = scr.tile([128, QPIX], F32, tag="t2",
                                      name=f"t2_{t}_{L}_{q}")
                        t1 = scr.tile([128, QPIX], F32, tag="t1",
                                      name=f"t1_{t}_{L}_{q}")
                        U = scr.tile([128, QPIX], F32, tag="u",
                                     name=f"u_{t}_{L}_{q}")
                        # sigmoid(o) = 0.5*tanh(o/2) + 0.5 (only needs TT)
                        nc.vector.tensor_scalar(U[64:128, :],
                                                TT[64:128, sl],
                                                0.5, 0.5,
                                                mybir.AluOpType.mult,
                                                mybir.AluOpType.add)
                        # i*tanh(g) (base0 -> out base64); f*c (base64)
                        nc.vector.tensor_mul(t2[64:128, :], SIG[0:64, sl],
                                             TT[0:64, sl])
                        nc.vector.tensor_mul(t1[64:128, :], SIG[64:128, sl],
                                             csl)
                        nc.vector.tensor_add(csl, t1[64:128, :],
                                             t2[64:128, :])
                        TC = work.tile([128, QPIX], F32, tag="tc",
                                       name=f"tc_{t}_{L}_{q}")
                        nc.scalar.activation(TC[64:128, :], csl, AF.Tanh)
                        rr = 1 + QROWS * q
                        if L == 0:
                            # write h0 straight into layer-1's input stack
                            hdst = HH1v[0:64, rr:rr + QROWS, 1:W + 1]
                        else:
                            hdst = HH1v[64:128, rr:rr + QROWS, 1:W + 1]
                        nc.vector.tensor_mul(
                            hdst,
                            U[64:128, :].rearrange("p (a b) -> p a b",
                                                   a=QROWS),
                            TC[64:128, :].rearrange("p (a b) -> p a b",
                                                    a=QROWS))
                        if L == 0 and t + 1 < T:
                            # copy back for layer-0's next step (off the
                            # critical path: not needed until t+1)
                            nc.sync.dma_start(
                                HH0v[0:64, rr:rr + QROWS, 1:W + 1],
                                HH1v[0:64, rr:rr + QROWS, 1:W + 1])
                    if L == 0 and t + 1 < T:
                        write_x(t + 1)
                    if L == 1:
                        nc.scalar.dma_start(
                            out1[t, :, :].rearrange("c (a b) -> c a b", a=H),
                            HH1v[64:128, 1:H + 1, 1:W + 1].bitcast(F32))
            nc.sync.dma_start(c1o[:, :], C1[64:128, :])
    nc.compile()
    return nc


def _host_prep(att_w1, att_w2, conv_w0, conv_b0, conv_w1, conv_b1):
    # output-channel permutation: [i, f, g, o]
    perm = np.concatenate([np.arange(0, 128), np.arange(192, 256),
                           np.arange(128, 192)])
    # layer0 lhsT: partition r<64 -> cin=3+r (h); r in 64..66 -> cin=r-64 (x)
    cin0 = np.concatenate([np.arange(3, 67), np.arange(0, 3)])
    w0p = conv_w0[perm][:, cin0]                    # [256, 67, 3, 3]
    w0l = np.ascontiguousarray(w0p.transpose(1, 2, 3, 0).reshape(67, 9, 256),
                               dtype=np.float32)
    w1p = conv_w1[perm]                             # [256, 128, 3, 3]
    w1l = np.ascontiguousarray(w1p.transpose(1, 2, 3, 0).reshape(128, 9, 256),
                               dtype=np.float32)
    b0p, b1p = conv_b0[perm], conv_b1[perm]
    bif0 = np.ascontiguousarray(b0p[0:128, None], np.float32)
    bgo0 = np.concatenate([b0p[128:192], 0.5 * b0p[192:256]])[:, None]
    bif1 = np.ascontiguousarray(b1p[0:128, None], np.float32)
    bgo1 = np.concatenate([b1p[128:192], 0.5 * b1p[192:256]])[:, None]

    w1mask = np.zeros((112, 32), np.float32)
    w2mask = np.zeros((32, 48), np.float32)
    for t in range(16):
        for c in range(3):
            w1mask[3 * t + c, t] = att_w1[0, c] / PIX      # fc1 on mean
            w1mask[64 + 3 * t + c, 16 + t] = att_w1[0, c]  # fc1 on max
            w2mask[t, 3 * t + c] = att_w2[c, 0]
            w2mask[16 + t, 3 * t + c] = att_w2[c, 0]
    return dict(w0=w0l, w1=w1l, bif0=bif0,
                bgo0=np.ascontiguousarray(bgo0, np.float32),
                bif1=bif1, bgo1=np.ascontiguousarray(bgo1, np.float32),
                w1m=w1mask, w2m=w2mask)


def run(inputs, trace=False):
    x = np.asarray(inputs["x"], np.float32)
    shared = _host_prep(np.asarray(inputs["att_w1"], np.float32),
                        np.asarray(inputs["att_w2"], np.float32),
                        np.asarray(inputs["conv_w0"], np.float32),
                        np.asarray(inputs["conv_b0"], np.float32),
                        np.asarray(inputs["conv_w1"], np.float32),
                        np.asarray(inputs["conv_b1"], np.float32))
    if "nc" not in _CACHE:
        _CACHE["nc"] = _build_nc()
    nc = _CACHE["nc"]
    in_maps = []
    for b in range(B):
        m = dict(shared)
        m["x48"] = np.ascontiguousarray(x[b].reshape(T * C, PIX))
        in_maps.append(m)
    res = run_bass_kernel_spmd(nc, in_maps, core_ids=list(range(B)),
                               trace=trace)
    out1 = np.stack([r["out1"].reshape(T, HID, H, W) for r in res.results])
    h1 = np.ascontiguousarray(out1[:, -1])
    c1 = np.stack([r["c1o"].reshape(HID, H, W) for r in res.results])
    return (out1, h1, c1), res


def kernel(**inputs):
    outs, _ = run(inputs, trace=bool(os.environ.get("TRN_KERNEL_TRACE")))
    return outs


# revision 9
# speedup vs baseline: 1.0590x; 1.0590x over previous
"""Trainium2 Bass kernel for nn_AttentionBasedConvLSTM.

Model: per-(b,t,c) SE-style channel attention gates x, then a 2-layer
ConvLSTM (hidden 64, 3x3 SAME convs) over T=16 steps on 64x64 frames.
Returns (out1, h1, c1) like the reference.

Sharding: data-parallel over batch B=8 across the 8 NeuronCores (one
batch element per core); weights replicated. Recurrent convs run as
9-tap accumulating matmuls on the tensor engine in float32r (~1e-4
rounding, full rate), gates via ScalarE sigmoid/tanh (sigmoid(o) is
0.5*tanh(o/2)+0.5 so one ACT op covers the mixed [g;o] half), cell/h
updates on VectorE in fp32.

Layouts (per core, SBUF):
  HH0 [128, 66*66] f32r: rows 0-63 h0 padded, rows 64-66 gated x_t
      padded -> layer-0 conv rhs is HH0[0:67] at 9 tap offsets (K=67
      keeps PE array occupancy >50% so the HAM clock stays at 2.4GHz).
  HH1 [128, 66*66] f32r: rows 0-63 h0(t), rows 64-127 h1(t-1)
      -> layer-1 conv rhs is HH1[0:128] (K=128).
  C0/C1 [128, 4096] fp32: rows 64-127 hold the cell state (base-64 so
      every elementwise op is partition-base aligned with the f/o gates).

Within a (t, layer): ALL conv matmuls are emitted before any h-write
(program order defines read-after-write semantics on the single-buffered
HH state); Tile's byte-range WAR tracking then orders the h-writes after
the tap reads of the neighboring rows.
"""
import os
import sys

sys.path.insert(0, "/opt/trn_rl_repo")

import numpy as np

import concourse.mybir as mybir
import concourse.tile as tile
from concourse import bacc
from concourse.bass_utils import run_bass_kernel_spmd

F32 = mybir.dt.float32
F32R = mybir.dt.float32r
AF = mybir.ActivationFunctionType

B, T, C, H, W = 8, 16, 3, 64, 64
HID = 64
HP, WP = H + 4, W + 2          # zero-padded frame (2 extra pad rows for
                               # the shifted x/h copies)
PADN = HP * WP
PIX = H * W                    # 4096
NCHUNK = 2                     # conv/psum chunks per frame (2048 px)
CHPIX = PIX // NCHUNK
CHROWS = H // NCHUNK           # 32 image rows per chunk
NSUB = CHPIX // 512            # 512-px matmul slices per chunk

_CACHE = {}


def _build_nc():
    nc = bacc.Bacc("TRN2", target_bir_lowering=False)

    x48 = nc.dram_tensor("x48", (T * C, PIX), F32, kind="ExternalInput")
    w0p = nc.dram_tensor("w0p", (128, 3, 256), F32R, kind="ExternalInput")
    w0dx = nc.dram_tensor("w0dx", (73, 3, 256), F32R, kind="ExternalInput")
    w1 = nc.dram_tensor("w1", (128, 9, 256), F32R, kind="ExternalInput")
    bif0 = nc.dram_tensor("bif0", (128, 1), F32, kind="ExternalInput")
    bgo0 = nc.dram_tensor("bgo0", (128, 1), F32, kind="ExternalInput")
    bif1 = nc.dram_tensor("bif1", (128, 1), F32, kind="ExternalInput")
    bgo1 = nc.dram_tensor("bgo1", (128, 1), F32, kind="ExternalInput")
    w1m = nc.dram_tensor("w1m", (112, 32), F32, kind="ExternalInput")
    w2m = nc.dram_tensor("w2m", (32, 48), F32, kind="ExternalInput")
    out1 = nc.dram_tensor("out1", (T, HID, PIX), F32, kind="ExternalOutput")
    c1o = nc.dram_tensor("c1o", (HID, PIX), F32, kind="ExternalOutput")

    with tile.TileContext(nc) as tc:
        with tc.tile_pool(name="const", bufs=1) as const, \
             tc.tile_pool(name="state", bufs=1) as state, \
             tc.tile_pool(name="work", bufs=2) as work, \
             tc.tile_pool(name="scr", bufs=1) as scr, \
             tc.tile_pool(name="psum", bufs=2, space="PSUM") as psum:

            # ---- loads (x first: the attention chain gates everything) --
            X48 = const.tile([T * C, PIX], F32)
            nc.sync.dma_start(X48[:], x48[:, :])
            W0P = const.tile([128, 3, 256], F32R)
            W0DX = const.tile([73, 3, 256], F32R)
            W1 = const.tile([128, 9, 256], F32R)
            nc.sync.dma_start(W0P[:], w0p[:, :, :])
            nc.sync.dma_start(W0DX[:], w0dx[:, :, :])
            nc.sync.dma_start(W1[:], w1[:, :, :])
            BIF0 = const.tile([128, 1], F32)
            BGO0 = const.tile([128, 1], F32)
            BIF1 = const.tile([128, 1], F32)
            BGO1 = const.tile([128, 1], F32)
            nc.scalar.dma_start(BIF0[:], bif0[:, :])
            nc.scalar.dma_start(BGO0[:], bgo0[:, :])
            nc.scalar.dma_start(BIF1[:], bif1[:, :])
            nc.scalar.dma_start(BGO1[:], bgo1[:, :])
            W1M = const.tile([112, 32], F32)
            W2M = const.tile([32, 48], F32)
            nc.scalar.dma_start(W1M[:], w1m[:, :])
            nc.scalar.dma_start(W2M[:], w2m[:, :])

            SCLV = const.tile([128, 1], F32)      # per-partition tanh scale
            nc.vector.memset(SCLV[0:64, :], 1.0)  # g rows
            nc.vector.memset(SCLV[64:128, :], 0.5)  # o rows
            ONES = const.tile([1, 1], F32)
            nc.vector.memset(ONES[:], 1.0)

            # ---- state -------------------------------------------------
            HH0 = state.tile([128, PADN], F32R)
            HH0B = state.tile([128, PADN], F32R)  # [h0; h0 shifted +1 row]
            HH1 = state.tile([128, PADN], F32R)
            C0 = state.tile([128, PIX], F32)
            C1 = state.tile([128, PIX], F32)
            nc.vector.memset(HH0[:].bitcast(F32), 0.0)
            nc.vector.memset(HH0B[:].bitcast(F32), 0.0)
            nc.vector.memset(HH1[:].bitcast(F32), 0.0)
            nc.vector.memset(C0[:], 0.0)
            nc.vector.memset(C1[:], 0.0)
            HH0v = HH0.rearrange("p (a b) -> p a b", a=HP)
            HH0Bv = HH0B.rearrange("p (a b) -> p a b", a=HP)
            HH1v = HH1.rearrange("p (a b) -> p a b", a=HP)

            # ---- attention: scale = sigmoid(fc(avg) + fc(max)) ---------
            V = const.tile([112, 1], F32)
            nc.vector.memset(V[:], 0.0)
            nc.vector.tensor_reduce(V[0:48, :], X48[:, :],
                                    axis=mybir.AxisListType.XYZW,
                                    op=mybir.AluOpType.add)
            nc.vector.tensor_reduce(V[64:112, :], X48[:, :],
                                    axis=mybir.AxisListType.XYZW,
                                    op=mybir.AluOpType.max)
            pa = psum.tile([128, CHPIX], F32, tag="gates", name="pa")
            nc.tensor.matmul(pa[0:1, 0:32], V[:, :], W1M[:, :],
                             start=True, stop=True)
            R = const.tile([1, 32], F32)
            nc.scalar.activation(R[:], pa[0:1, 0:32], AF.Relu)
            pb = psum.tile([128, CHPIX], F32, tag="gates", name="pb")
            nc.tensor.matmul(pb[0:32, 0:1], R[:, :], ONES[:, :],
                             start=True, stop=True)
            R2 = const.tile([32, 1], F32)
            nc.vector.tensor_copy(R2[:], pb[0:32, 0:1])
            pc = psum.tile([128, CHPIX], F32, tag="gates", name="pc")
            nc.tensor.matmul(pc[0:48, 0:1], W2M[:, :], R2[:, :],
                             start=True, stop=True)
            SCL = const.tile([48, 1], F32)
            nc.scalar.activation(SCL[:], pc[0:48, 0:1], AF.Sigmoid)
            XIN = const.tile([48, PIX], F32R)
            nc.vector.tensor_scalar(XIN[:], X48[:, :], SCL[:, 0:1], None,
                                    mybir.AluOpType.mult)
            XINv = XIN.rearrange("p (a b) -> p a b", a=H)

            def write_x(t):
                # gated x_t at 3 row-shifts: rows 64+3s hold x shifted by
                # s pad rows, so the dy=2 tap view delivers taps (2-s, dx)
                for sshift in range(3):
                    nc.sync.dma_start(
                        HH0v[64 + 3 * sshift:67 + 3 * sshift,
                             1 + sshift:H + 1 + sshift, 1:W + 1],
                        XINv[3 * t:3 * t + 3, :, :])

            write_x(0)

            # ---- recurrence --------------------------------------------
            layers = (
                (HH0v, 67, None, BIF0, BGO0, C0),
                (HH1v, 128, W1, BIF1, BGO1, C1),
            )
            for t in range(T):
                for L in (0, 1):
                    HHtv, K, Wl, Bif, Bgo, Cc = layers[L]
                    # Phase 1: ALL conv matmuls (both chunks) + gate ACTs.
                    sigs, tts = [], []
                    for ch in range(NCHUNK):
                        ps0 = psum.tile([128, CHPIX], F32, tag="gates",
                                        name=f"ps0_{t}_{L}_{ch}")
                        ps1 = psum.tile([128, CHPIX], F32, tag="gates",
                                        name=f"ps1_{t}_{L}_{ch}")
                        for half, ps in ((0, ps0), (1, ps1)):
                            hs = slice(128 * half, 128 * (half + 1))
                            if L == 1:
                                for tap in range(9):
                                    dy, dx = tap // 3, tap % 3
                                    for sub in range(NSUB):
                                        r0 = CHROWS * ch + 8 * sub
                                        rhs = HHtv[0:128,
                                                   r0 + dy:r0 + dy + 8,
                                                   dx:dx + W]
                                        nc.tensor.matmul(
                                            ps[:, 512 * sub:512 * (sub + 1)],
                                            Wl[0:128, tap, hs],
                                            rhs, start=(tap == 0),
                                            stop=(tap == 8))
                            else:
                                # 3x K=73 [h-tap(2,dx); x all taps] + 3x
                                # K=128 delta pairs [h-tap(1,dx); h-tap(0,dx)]
                                for dx in range(3):
                                    for sub in range(NSUB):
                                        r0 = CHROWS * ch + 8 * sub
                                        nc.tensor.matmul(
                                            ps[:, 512 * sub:512 * (sub + 1)],
                                            W0DX[0:73, dx, hs],
                                            HH0v[0:73, r0 + 2:r0 + 10,
                                                 dx:dx + W],
                                            start=(dx == 0), stop=False,
                                            skip_group_check=True)
                                for dx in range(3):
                                    for sub in range(NSUB):
                                        r0 = CHROWS * ch + 8 * sub
                                        nc.tensor.matmul(
                                            ps[:, 512 * sub:512 * (sub + 1)],
                                            W0P[0:128, dx, hs],
                                            HH0Bv[0:128, r0 + 1:r0 + 9,
                                                  dx:dx + W],
                                            start=False, stop=(dx == 2),
                                            skip_group_check=True)
                        # gates: ps0=[i;f] sigmoid; ps1=[g;o] tanh w/ scale
                        SIG = work.tile([128, CHPIX], F32, tag="sig",
                                        name=f"sig_{t}_{L}_{ch}")
                        TT = work.tile([128, CHPIX], F32, tag="tt",
                                       name=f"tt_{t}_{L}_{ch}")
                        nc.scalar.activation(SIG[:], ps0[:], AF.Sigmoid,
                                             bias=Bif[:, 0:1])
                        nc.scalar.activation(TT[:], ps1[:], AF.Tanh,
                                             bias=Bgo[:, 0:1],
                                             scale=SCLV[:, 0:1])
                        sigs.append(SIG)
                        tts.append(TT)
                    # Phase 2: cell/h updates in 1024-px quarters so h
                    # lands progressively (shortens the PE boundary stall).
                    NQ = 4
                    QPIX = PIX // NQ
                    QROWS = H // NQ
                    for q in range(NQ):
                        ch, qq = q // 2, q % 2
                        SIG, TT = sigs[ch], tts[ch]
                        sl = slice(QPIX * qq, QPIX * (qq + 1))
                        csl = Cc[64:128, QPIX * q:QPIX * (q + 1)]
                        t2 = scr.tile([128, QPIX], F32, tag="t2",
                                      name=f"t2_{t}_{L}_{q}")
                        t1 = scr.tile([128, QPIX], F32, tag="t1",
                                      name=f"t1_{t}_{L}_{q}")
                        U = scr.tile([128, QPIX], F32, tag="u",
                                     name=f"u_{t}_{L}_{q}")
                        # sigmoid(o) = 0.5*tanh(o/2) + 0.5 (only needs TT)
                        nc.vector.tensor_scalar(U[64:128, :],
                                                TT[64:128, sl],
                                                0.5, 0.5,
                                                mybir.AluOpType.mult,
                                                mybir.AluOpType.add)
                        # i*tanh(g) (base0 -> out base64); f*c (base64)
                        nc.vector.tensor_mul(t2[64:128, :], SIG[0:64, sl],
                                             TT[0:64, sl])
                        nc.vector.tensor_mul(t1[64:128, :], SIG[64:128, sl],
                                             csl)
                        nc.vector.tensor_add(csl, t1[64:128, :],
                                             t2[64:128, :])
                        TC = work.tile([128, QPIX], F32, tag="tc",
                                       name=f"tc_{t}_{L}_{q}")
                        nc.scalar.activation(TC[64:128, :], csl, AF.Tanh)
                        rr = 1 + QROWS * q
                        if L == 0:
                            # write h0 straight into layer-1's input stack
                            hdst = HH1v[0:64, rr:rr + QROWS, 1:W + 1]
                        else:
                            hdst = HH1v[64:128, rr:rr + QROWS, 1:W + 1]
                        nc.vector.tensor_mul(
                            hdst,
                            U[64:128, :].rearrange("p (a b) -> p a b",
                                                   a=QROWS),
                            TC[64:128, :].rearrange("p (a b) -> p a b",
                                                    a=QROWS))
                        if L == 0 and t + 1 < T:
                            # copy back for layer-0's next step (off the
                            # critical path: not needed until t+1)
                            nc.sync.dma_start(
                                HH0v[0:64, rr:rr + QROWS, 1:W + 1],
                                HH1v[0:64, rr:rr + QROWS, 1:W + 1])
                            nc.sync.dma_start(
                                HH0Bv[0:64, rr:rr + QROWS, 1:W + 1],
                                HH1v[0:64, rr:rr + QROWS, 1:W + 1])
                            nc.sync.dma_start(
                                HH0Bv[64:128, rr + 1:rr + 1 + QROWS,
                                      1:W + 1],
                                HH1v[0:64, rr:rr + QROWS, 1:W + 1])
                    if L == 0 and t + 1 < T:
                        write_x(t + 1)
                    if L == 1:
                        nc.scalar.dma_start(
                            out1[t, :, :].rearrange("c (a b) -> c a b", a=H),
                            HH1v[64:128, 1:H + 1, 1:W + 1].bitcast(F32))
            nc.sync.dma_start(c1o[:, :], C1[64:128, :])
    nc.compile()
    return nc


def _host_prep(att_w1, att_w2, conv_w0, conv_b0, conv_w1, conv_b1):
    # output-channel permutation: [i, f, g, o]
    perm = np.concatenate([np.arange(0, 128), np.arange(192, 256),
                           np.arange(128, 192)])
    # layer0: h-part [64ch] and x-part [3ch] lhsT layouts
    w0h = conv_w0[perm][:, 3:67]                    # [256, 64, 3, 3]
    w0hl = w0h.transpose(1, 2, 3, 0).reshape(64, 9, 256)   # [64, tap, 256]
    w0xw = conv_w0[perm][:, 0:3]                    # [256, 3, 3, 3]
    w0xl = w0xw.transpose(1, 2, 3, 0).reshape(3, 9, 256)   # [3, tap, 256]
    w0pair = np.empty((128, 3, 256), np.float32)
    w0dx = np.empty((73, 3, 256), np.float32)
    for dx in range(3):
        w0pair[0:64, dx, :] = w0hl[:, 3 + dx, :]    # tap (1,dx) low
        w0pair[64:128, dx, :] = w0hl[:, dx, :]      # tap (0,dx) high
        w0dx[0:64, dx, :] = w0hl[:, 6 + dx, :]      # h tap (2,dx)
        w0dx[64:67, dx, :] = w0xl[:, 6 + dx, :]     # x tap (2,dx), shift 0
        w0dx[67:70, dx, :] = w0xl[:, 3 + dx, :]     # x tap (1,dx), shift 1
        w0dx[70:73, dx, :] = w0xl[:, dx, :]         # x tap (0,dx), shift 2
    w1p = conv_w1[perm]                             # [256, 128, 3, 3]
    w1l = np.ascontiguousarray(w1p.transpose(1, 2, 3, 0).reshape(128, 9, 256),
                               dtype=np.float32)
    b0p, b1p = conv_b0[perm], conv_b1[perm]
    bif0 = np.ascontiguousarray(b0p[0:128, None], np.float32)
    bgo0 = np.concatenate([b0p[128:192], 0.5 * b0p[192:256]])[:, None]
    bif1 = np.ascontiguousarray(b1p[0:128, None], np.float32)
    bgo1 = np.concatenate([b1p[128:192], 0.5 * b1p[192:256]])[:, None]

    w1mask = np.zeros((112, 32), np.float32)
    w2mask = np.zeros((32, 48), np.float32)
    for t in range(16):
        for c in range(3):
            w1mask[3 * t + c, t] = att_w1[0, c] / PIX      # fc1 on mean
            w1mask[64 + 3 * t + c, 16 + t] = att_w1[0, c]  # fc1 on max
            w2mask[t, 3 * t + c] = att_w2[c, 0]
            w2mask[16 + t, 3 * t + c] = att_w2[c, 0]
    return dict(w0p=np.ascontiguousarray(w0pair),
                w0dx=np.ascontiguousarray(w0dx), w1=w1l, bif0=bif0,
                bgo0=np.ascontiguousarray(bgo0, np.float32),
                bif1=bif1, bgo1=np.ascontiguousarray(bgo1, np.float32),
                w1m=w1mask, w2m=w2mask)


def run(inputs, trace=False):
    x = np.asarray(inputs["x"], np.float32)
    shared = _host_prep(np.asarray(inputs["att_w1"], np.float32),
                        np.asarray(inputs["att_w2"], np.float32),
                        np.asarray(inputs["conv_w0"], np.float32),
                        np.asarray(inputs["conv_b0"], np.float32),
                        np.asarray(inputs["conv_w1"], np.float32),
                        np.asarray(inputs["conv_b1"], np.float32))
    if "nc" not in _CACHE:
        _CACHE["nc"] = _build_nc()
    nc = _CACHE["nc"]
    in_maps = []
    for b in range(B):
        m = dict(shared)
        m["x48"] = np.ascontiguousarray(x[b].reshape(T * C, PIX))
        in_maps.append(m)
    res = run_bass_kernel_spmd(nc, in_maps, core_ids=list(range(B)),
                               trace=trace)
    out1 = np.stack([r["out1"].reshape(T, HID, H, W) for r in res.results])
    h1 = np.ascontiguousarray(out1[:, -1])
    c1 = np.stack([r["c1o"].reshape(HID, H, W) for r in res.results])
    return (out1, h1, c1), res


def kernel(**inputs):
    outs, _ = run(inputs, trace=bool(os.environ.get("TRN_KERNEL_TRACE")))
    return outs
